# revision 2
# baseline (speedup 1.0000x reference)
"""MoE grouped-GEMM kernel for Trainium2 (8 NeuronCores, expert-parallel).

Problem: x [16384, 1024] fp16, expert_indices [16384] int32 (0..7),
weights [8, 1024, 4096] fp16. Output: fp16 [16384, 4096] in sorted-token
order (stable sort by expert), fp32 accumulation.

Sharding: the host performs the argsort/bincount dispatch (that IS the
sharding step) and gives core e the tokens routed to expert e as a
pre-transposed xT [K, Mpad] fp16 block plus that expert's weights
[K, N]. Every core runs the identical dense-GEMM program (token counts
padded to a common multiple of 128), so a single SPMD NEFF drives all 8
cores with no device-side collectives. The host concatenates the
per-expert output blocks, which is exactly sorted-token order.
"""

import numpy as np

_NCORES = 8


def _build_program(T, K, N):
    """Dense GEMM per core: out[Mpad, N] = xT.T @ w, fp32 PSUM accumulation.

    Layout per core:
      xT [K, Mpad] fp16  (x pre-transposed on host so K lands on partitions)
      w  [K, N]   fp16
      out [Mpad, N] fp16, Mpad = T*128

    PE mapping: stationary lhsT = xT k-tile [128, 128], moving rhs = w
    [128, 512] slice, PSUM [128m, 512n] fp32 accumulated over K/128
    k-tiles. PSUM is split into two 4-bank halves (bufs=2) so the
    DVE fp32->fp16 eviction of one half overlaps matmuls in the other.
    """
    from concourse import bacc, bass, tile
    import concourse.mybir as mybir

    f16 = mybir.dt.float16
    f32 = mybir.dt.float32
    Mpad = T * 128
    KT = K // 128            # k-tiles (contraction)
    NB = 512                 # one PSUM bank of fp32
    NH = 2048                # psum half (4 banks)
    nhalves = N // NH

    nc = bacc.Bacc(
        "TRN2", target_bir_lowering=False, debug=False, num_devices=_NCORES
    )
    xT = nc.dram_tensor("xT", [K, Mpad], f16, kind="ExternalInput").ap()
    w = nc.dram_tensor("w", [K, N], f16, kind="ExternalInput").ap()
    out = nc.dram_tensor("out", [Mpad, N], f16, kind="ExternalOutput").ap()

    with tile.TileContext(nc) as tc:
        with (
            tc.tile_pool(name="xw", bufs=1) as xw,
            tc.tile_pool(name="op", bufs=4) as op,
            tc.tile_pool(name="pp", bufs=2, space=bass.MemorySpace.PSUM) as pp,
        ):
            # Whole x and w stay SBUF-resident (~99KB/partition total).
            # Loads are chunked (w: one tile per 512-col bank slice, x: a
            # small t=0 head + the rest) and issued in first-use order so
            # the t=0 accumulation starts ~1-2us in and strip k arrives
            # before the k-loop needs it; a monolithic load order left the
            # PE idle for ~14us at startup (round-robin DMA queues).
            xheads = []
            xrests = []
            # wt[h][k][n] -> [128, 512] tile
            wt = [
                [[None] * (NH // NB) for _ in range(KT)] for _ in range(nhalves)
            ]
            for k in range(KT):
                xh = xw.tile([128, 128], f16, tag=f"xh{k}")
                nc.sync.dma_start(xh[:], xT[k * 128 : (k + 1) * 128, 0:128])
                xheads.append(xh)
                for n in range(NH // NB):
                    c = xw.tile([128, NB], f16, tag=f"w0_{k}_{n}")
                    nc.sync.dma_start(
                        c[:], w[k * 128 : (k + 1) * 128, n * NB : (n + 1) * NB]
                    )
                    wt[0][k][n] = c
            for k in range(KT):
                xr = xw.tile([128, Mpad - 128], f16, tag=f"xr{k}")
                nc.sync.dma_start(xr[:], xT[k * 128 : (k + 1) * 128, 128:Mpad])
                xrests.append(xr)
                for h in range(1, nhalves):
                    for n in range(NH // NB):
                        c = xw.tile([128, NB], f16, tag=f"w{h}_{k}_{n}")
                        nc.sync.dma_start(
                            c[:],
                            w[
                                k * 128 : (k + 1) * 128,
                                h * NH + n * NB : h * NH + (n + 1) * NB,
                            ],
                        )
                        wt[h][k][n] = c

            for t in range(T):
                for h in range(nhalves):
                    ps = pp.tile([128, NH], f32, tag="ps")
                    for k in range(KT):
                        if t == 0:
                            lhs = xheads[k][:]
                        else:
                            lhs = xrests[k][:, (t - 1) * 128 : t * 128]
                        for n in range(NH // NB):
                            nc.tensor.matmul(
                                ps[:, n * NB : (n + 1) * NB],
                                lhs,
                                wt[h][k][n][:],
                                start=(k == 0),
                                stop=(k == KT - 1),
                            )
                    ot = op.tile([128, NH], f16, tag="ot")
                    nc.vector.tensor_copy(ot[:], ps[:])
                    nc.sync.dma_start(
                        out[t * 128 : (t + 1) * 128, h * NH : (h + 1) * NH], ot[:]
                    )
    nc.compile()
    return nc


# test.py reads these after a call for timing/trace introspection
last_results = None


def kernel(x, expert_indices, weights):
    x = np.asarray(x)
    ei = np.asarray(expert_indices)
    w = np.asarray(weights)
    M, K = x.shape
    E, K2, N = w.shape
    assert K == K2 and E == _NCORES

    counts = np.bincount(ei, minlength=E)
    T = max(1, -(-int(counts.max()) // 128))
    Mpad = T * 128
    order = np.argsort(ei, kind="stable")
    x_sorted = x[order]
    offs = np.zeros(E + 1, dtype=np.int64)
    np.cumsum(counts, out=offs[1:])

    in_maps = []
    for e in range(E):
        blk = x_sorted[offs[e] : offs[e + 1]]
        xeT = np.zeros((K, Mpad), dtype=np.float16)
        xeT[:, : blk.shape[0]] = blk.T
        in_maps.append({"xT": xeT, "w": np.ascontiguousarray(w[e])})

    nc = _build_program(T, K, N)

    from concourse.bass_utils import run_bass_kernel_spmd

    res = run_bass_kernel_spmd(nc, in_maps, list(range(E)))
    global last_results
    last_results = res

    out = np.empty((M, N), dtype=np.float16)
    for e in range(E):
        out[offs[e] : offs[e + 1]] = res.results[e]["out"][: counts[e]]
    return out


# revision 3
# speedup vs baseline: 1.2558x; 1.2558x over previous
"""MoE grouped-GEMM kernel for Trainium2 (8 NeuronCores, expert-parallel).

Problem: x [16384, 1024] fp16, expert_indices [16384] int32 (0..7),
weights [8, 1024, 4096] fp16. Output: fp16 [16384, 4096] in sorted-token
order (stable sort by expert), fp32 accumulation.

Sharding: the host performs the argsort/bincount dispatch (that IS the
sharding step) and gives core e the tokens routed to expert e as a
pre-transposed xT [K, Mpad] fp16 block plus that expert's weights
[K, N]. Every core runs the identical dense-GEMM program (token counts
padded to a common multiple of 128), so a single SPMD NEFF drives all 8
cores with no device-side collectives. The host concatenates the
per-expert output blocks, which is exactly sorted-token order.
"""

import numpy as np

_NCORES = 8


def _build_program(T, K, N):
    """Dense GEMM per core: out[Mpad, N] = xT.T @ w, fp32 PSUM accumulation.

    Layout per core:
      xT [K, Mpad] fp16  (x pre-transposed on host so K lands on partitions)
      w  [K, N]   fp16
      out [Mpad, N] fp16, Mpad = T*128

    PE mapping: stationary lhsT = xT k-tile [128, 128], moving rhs = w
    [128, 512] slice, PSUM [128m, 512n] fp32 accumulated over K/128
    k-tiles. PSUM is split into two 4-bank halves (bufs=2) so the
    DVE fp32->fp16 eviction of one half overlaps matmuls in the other.
    """
    from concourse import bacc, bass, tile
    import concourse.mybir as mybir

    f16 = mybir.dt.float16
    f32 = mybir.dt.float32
    Mpad = T * 128
    KT = K // 128            # k-tiles (contraction)
    NB = 512                 # one PSUM bank of fp32
    NH = 2048                # psum half (4 banks)
    nhalves = N // NH

    nc = bacc.Bacc(
        "TRN2", target_bir_lowering=False, debug=False, num_devices=_NCORES
    )
    xT = nc.dram_tensor("xT", [K, Mpad], f16, kind="ExternalInput").ap()
    w = nc.dram_tensor("w", [K, N], f16, kind="ExternalInput").ap()
    out = nc.dram_tensor("out", [Mpad, N], f16, kind="ExternalOutput").ap()

    with tile.TileContext(nc) as tc:
        with (
            tc.tile_pool(name="xw", bufs=1) as xw,
            tc.tile_pool(name="op", bufs=4) as op,
            tc.tile_pool(name="pp", bufs=2, space=bass.MemorySpace.PSUM) as pp,
        ):
            # Whole x and w stay SBUF-resident (~99KB/partition total).
            # Rail split: x strips ride gpsimd SWDGE, w strips ride the
            # sync HWDGE ring, outputs ride the scalar HWDGE ring, so
            # first-use loads are never FIFO'd behind later traffic.
            # h-outer loop order means phase h=0 only needs the first
            # 2048 w columns (4MB) to reach steady state; h=1 strips
            # stream in during the ~115us h=0 phase.
            xs = []
            ws = [[None] * nhalves for _ in range(KT)]
            for k in range(KT):
                xt = xw.tile([128, Mpad], f16, tag=f"x{k}")
                nc.gpsimd.dma_start(xt[:], xT[k * 128 : (k + 1) * 128, :])
                xs.append(xt)
                wt = xw.tile([128, NH], f16, tag=f"w{k}h0")
                nc.sync.dma_start(wt[:], w[k * 128 : (k + 1) * 128, 0:NH])
                ws[k][0] = wt
            for h in range(1, nhalves):
                for k in range(KT):
                    wt = xw.tile([128, NH], f16, tag=f"w{k}h{h}")
                    nc.sync.dma_start(
                        wt[:], w[k * 128 : (k + 1) * 128, h * NH : (h + 1) * NH]
                    )
                    ws[k][h] = wt

            for h in range(nhalves):
                for t in range(T):
                    ps = pp.tile([128, NH], f32, tag="ps")
                    for k in range(KT):
                        lhs = xs[k][:, t * 128 : (t + 1) * 128]
                        for n in range(NH // NB):
                            nc.tensor.matmul(
                                ps[:, n * NB : (n + 1) * NB],
                                lhs,
                                ws[k][h][:, n * NB : (n + 1) * NB],
                                start=(k == 0),
                                stop=(k == KT - 1),
                            )
                    ot = op.tile([128, NH], f16, tag="ot")
                    nc.vector.tensor_copy(ot[:], ps[:])
                    nc.scalar.dma_start(
                        out[t * 128 : (t + 1) * 128, h * NH : (h + 1) * NH], ot[:]
                    )
    nc.compile()
    return nc


# test.py reads these after a call for timing/trace introspection
last_results = None


def kernel(x, expert_indices, weights):
    x = np.asarray(x)
    ei = np.asarray(expert_indices)
    w = np.asarray(weights)
    M, K = x.shape
    E, K2, N = w.shape
    assert K == K2 and E == _NCORES

    counts = np.bincount(ei, minlength=E)
    T = max(1, -(-int(counts.max()) // 128))
    Mpad = T * 128
    order = np.argsort(ei, kind="stable")
    x_sorted = x[order]
    offs = np.zeros(E + 1, dtype=np.int64)
    np.cumsum(counts, out=offs[1:])

    in_maps = []
    for e in range(E):
        blk = x_sorted[offs[e] : offs[e + 1]]
        xeT = np.zeros((K, Mpad), dtype=np.float16)
        xeT[:, : blk.shape[0]] = blk.T
        in_maps.append({"xT": xeT, "w": np.ascontiguousarray(w[e])})

    nc = _build_program(T, K, N)

    from concourse.bass_utils import run_bass_kernel_spmd

    res = run_bass_kernel_spmd(nc, in_maps, list(range(E)))
    global last_results
    last_results = res

    out = np.empty((M, N), dtype=np.float16)
    for e in range(E):
        out[offs[e] : offs[e + 1]] = res.results[e]["out"][: counts[e]]
    return out


# revision 4
# speedup vs baseline: 1.2711x; 1.0122x over previous
"""MoE grouped-GEMM kernel for Trainium2 (8 NeuronCores, expert-parallel).

Problem: x [16384, 1024] fp16, expert_indices [16384] int32 (0..7),
weights [8, 1024, 4096] fp16. Output: fp16 [16384, 4096] in sorted-token
order (stable sort by expert), fp32 accumulation.

Sharding: the host performs the argsort/bincount dispatch (that IS the
sharding step) and gives core e the tokens routed to expert e as a
pre-transposed xT [K, Mpad] fp16 block plus that expert's weights
[K, N]. Every core runs the identical dense-GEMM program (token counts
padded to a common multiple of 128), so a single SPMD NEFF drives all 8
cores with no device-side collectives. The host concatenates the
per-expert output blocks, which is exactly sorted-token order.
"""

import numpy as np

_NCORES = 8


def _build_program(T, K, N):
    """Dense GEMM per core: out[Mpad, N] = xT.T @ w, fp32 PSUM accumulation.

    Layout per core:
      xT [K, Mpad] fp16  (x pre-transposed on host so K lands on partitions)
      w  [K, N]   fp16
      out [Mpad, N] fp16, Mpad = T*128

    PE mapping: stationary lhsT = xT k-tile [128, 128], moving rhs = w
    [128, 512] slice, PSUM [128m, 512n] fp32 accumulated over K/128
    k-tiles. PSUM is split into two 4-bank halves (bufs=2) so the
    DVE fp32->fp16 eviction of one half overlaps matmuls in the other.
    """
    from concourse import bacc, bass, tile
    import concourse.mybir as mybir

    f16 = mybir.dt.float16
    f32 = mybir.dt.float32
    Mpad = T * 128
    KT = K // 128            # k-tiles (contraction)
    NB = 512                 # one PSUM bank of fp32
    NH = 2048                # psum half (4 banks)
    nhalves = N // NH

    nc = bacc.Bacc(
        "TRN2", target_bir_lowering=False, debug=False, num_devices=_NCORES
    )
    xT = nc.dram_tensor("xT", [K, Mpad], f16, kind="ExternalInput").ap()
    w = nc.dram_tensor("w", [K, N], f16, kind="ExternalInput").ap()
    out = nc.dram_tensor("out", [Mpad, N], f16, kind="ExternalOutput").ap()

    with tile.TileContext(nc) as tc:
        with (
            tc.tile_pool(name="xw", bufs=1) as xw,
            tc.tile_pool(name="op", bufs=4) as op,
            tc.tile_pool(name="pp", bufs=2, space=bass.MemorySpace.PSUM) as pp,
        ):
            # Whole x and w stay SBUF-resident (~99KB/partition total).
            # Rail split: x strips ride gpsimd SWDGE, w strips ride the
            # sync HWDGE ring, outputs ride the scalar HWDGE ring, so
            # first-use loads are never FIFO'd behind later traffic.
            # h-outer loop order means phase h=0 only needs the first
            # 2048 w columns (4MB) to reach steady state; h=1 strips
            # stream in during the ~115us h=0 phase.
            xs = []
            ws = [[None] * nhalves for _ in range(KT)]
            # x strip 0 rides the fast sync rail ahead of w so the first
            # matmul's deps land earliest; remaining x strips go via
            # gpsimd SWDGE (separate rail, needed within ~20us).
            x0 = xw.tile([128, Mpad], f16, tag="x0")
            nc.sync.dma_start(x0[:], xT[0:128, :])
            xs.append(x0)
            for k in range(KT):
                wt = xw.tile([128, NH], f16, tag=f"w{k}h0")
                nc.sync.dma_start(wt[:], w[k * 128 : (k + 1) * 128, 0:NH])
                ws[k][0] = wt
            for k in range(1, KT):
                xt = xw.tile([128, Mpad], f16, tag=f"x{k}")
                nc.gpsimd.dma_start(xt[:], xT[k * 128 : (k + 1) * 128, :])
                xs.append(xt)
            for h in range(1, nhalves):
                for k in range(KT):
                    wt = xw.tile([128, NH], f16, tag=f"w{k}h{h}")
                    nc.sync.dma_start(
                        wt[:], w[k * 128 : (k + 1) * 128, h * NH : (h + 1) * NH]
                    )
                    ws[k][h] = wt

            NQ = 1024  # output eviction chunk (cast + store pipelined)
            for h in range(nhalves):
                for t in range(T):
                    ps = pp.tile([128, NH], f32, tag="ps")
                    for k in range(KT):
                        lhs = xs[k][:, t * 128 : (t + 1) * 128]
                        for n in range(NH // NB):
                            nc.tensor.matmul(
                                ps[:, n * NB : (n + 1) * NB],
                                lhs,
                                ws[k][h][:, n * NB : (n + 1) * NB],
                                start=(k == 0),
                                stop=(k == KT - 1),
                            )
                    for q in range(NH // NQ):
                        ot = op.tile([128, NQ], f16, tag="ot")
                        nc.vector.tensor_copy(ot[:], ps[:, q * NQ : (q + 1) * NQ])
                        # alternate output rails (sync HWDGE ran ~105GB/s,
                        # scalar ~80GB/s; either alone barely keeps up)
                        eng = nc.scalar if (t * nhalves + h + q) % 2 else nc.sync
                        eng.dma_start(
                            out[
                                t * 128 : (t + 1) * 128,
                                h * NH + q * NQ : h * NH + (q + 1) * NQ,
                            ],
                            ot[:],
                        )
    nc.compile()
    return nc


# test.py reads these after a call for timing/trace introspection
last_results = None


def kernel(x, expert_indices, weights):
    x = np.asarray(x)
    ei = np.asarray(expert_indices)
    w = np.asarray(weights)
    M, K = x.shape
    E, K2, N = w.shape
    assert K == K2 and E == _NCORES

    counts = np.bincount(ei, minlength=E)
    T = max(1, -(-int(counts.max()) // 128))
    Mpad = T * 128
    order = np.argsort(ei, kind="stable")
    x_sorted = x[order]
    offs = np.zeros(E + 1, dtype=np.int64)
    np.cumsum(counts, out=offs[1:])

    in_maps = []
    for e in range(E):
        blk = x_sorted[offs[e] : offs[e + 1]]
        xeT = np.zeros((K, Mpad), dtype=np.float16)
        xeT[:, : blk.shape[0]] = blk.T
        in_maps.append({"xT": xeT, "w": np.ascontiguousarray(w[e])})

    nc = _build_program(T, K, N)

    from concourse.bass_utils import run_bass_kernel_spmd

    res = run_bass_kernel_spmd(nc, in_maps, list(range(E)))
    global last_results
    last_results = res

    out = np.empty((M, N), dtype=np.float16)
    for e in range(E):
        out[offs[e] : offs[e + 1]] = res.results[e]["out"][: counts[e]]
    return out


# revision 6
# speedup vs baseline: 1.2750x; 1.0030x over previous
"""MoE grouped-GEMM kernel for Trainium2 (8 NeuronCores, expert-parallel).

Problem: x [16384, 1024] fp16, expert_indices [16384] int32 (0..7),
weights [8, 1024, 4096] fp16. Output: fp16 [16384, 4096] in sorted-token
order (stable sort by expert), fp32 accumulation.

Sharding: the host performs the argsort/bincount dispatch (that IS the
sharding step) and gives core e the tokens routed to expert e as a
pre-transposed xT [K, Mpad] fp16 block plus that expert's weights
[K, N]. Every core runs the identical dense-GEMM program (token counts
padded to a common multiple of 128), so a single SPMD NEFF drives all 8
cores with no device-side collectives. The host concatenates the
per-expert output blocks, which is exactly sorted-token order.
"""

import numpy as np

_NCORES = 8


def _build_program(T, K, N):
    """Dense GEMM per core: out[Mpad, N] = xT.T @ w, fp32 PSUM accumulation.

    Layout per core:
      xT [K, Mpad] fp16  (x pre-transposed on host so K lands on partitions)
      w  [K, N]   fp16
      out [Mpad, N] fp16, Mpad = T*128

    PE mapping: stationary lhsT = xT k-tile [128, 128], moving rhs = w
    [128, 512] slice, PSUM [128m, 512n] fp32 accumulated over K/128
    k-tiles. PSUM is split into two 4-bank halves (bufs=2) so the
    DVE fp32->fp16 eviction of one half overlaps matmuls in the other.
    """
    from concourse import bacc, bass, tile
    import concourse.mybir as mybir

    f16 = mybir.dt.float16
    f32 = mybir.dt.float32
    Mpad = T * 128
    KT = K // 128            # k-tiles (contraction)
    NB = 512                 # one PSUM bank of fp32
    NH = 2048                # psum half (4 banks)
    nhalves = N // NH

    nc = bacc.Bacc(
        "TRN2", target_bir_lowering=False, debug=False, num_devices=_NCORES
    )
    xT = nc.dram_tensor("xT", [K, Mpad], f16, kind="ExternalInput").ap()
    w = nc.dram_tensor("w", [K, N], f16, kind="ExternalInput").ap()
    out = nc.dram_tensor("out", [Mpad, N], f16, kind="ExternalOutput").ap()

    with tile.TileContext(nc) as tc:
        with (
            tc.tile_pool(name="xw", bufs=1) as xw,
            tc.tile_pool(name="op", bufs=4) as op,
            tc.tile_pool(name="pp", bufs=2, space=bass.MemorySpace.PSUM) as pp,
        ):
            # Whole x and w stay SBUF-resident (~99KB/partition total).
            # Rail split: x strips ride gpsimd SWDGE, w strips ride the
            # sync HWDGE ring, outputs ride the scalar HWDGE ring, so
            # first-use loads are never FIFO'd behind later traffic.
            # h-outer loop order means phase h=0 only needs the first
            # 2048 w columns (4MB) to reach steady state; h=1 strips
            # stream in during the ~115us h=0 phase.
            xs = []
            ws = [[None] * nhalves for _ in range(KT)]
            # All input strips ride the sync HWDGE rail (it alone reaches
            # ~400GB/s; a second concurrent rail just splits HBM bandwidth
            # and delays first-use strips). Issue in exact first-use order:
            # (x_k, w_k_h0) pairs feed the h=0 k-loop, then the h=1 strips
            # which aren't needed until ~115us in.
            for k in range(KT):
                xt = xw.tile([128, Mpad], f16, tag=f"x{k}")
                nc.sync.dma_start(xt[:], xT[k * 128 : (k + 1) * 128, :])
                xs.append(xt)
                wt = xw.tile([128, NH], f16, tag=f"w{k}h0")
                nc.sync.dma_start(wt[:], w[k * 128 : (k + 1) * 128, 0:NH])
                ws[k][0] = wt
            for h in range(1, nhalves):
                for k in range(KT):
                    wt = xw.tile([128, NH], f16, tag=f"w{k}h{h}")
                    nc.sync.dma_start(
                        wt[:], w[k * 128 : (k + 1) * 128, h * NH : (h + 1) * NH]
                    )
                    ws[k][h] = wt

            NQ = 1024  # output eviction chunk (cast + store pipelined)
            for h in range(nhalves):
                for t in range(T):
                    ps = pp.tile([128, NH], f32, tag="ps")
                    for k in range(KT):
                        lhs = xs[k][:, t * 128 : (t + 1) * 128]
                        for n in range(NH // NB):
                            nc.tensor.matmul(
                                ps[:, n * NB : (n + 1) * NB],
                                lhs,
                                ws[k][h][:, n * NB : (n + 1) * NB],
                                start=(k == 0),
                                stop=(k == KT - 1),
                            )
                    # finer chunks on the very last eviction shorten the tail
                    nq = 512 if (h == nhalves - 1 and t == T - 1) else NQ
                    for q in range(NH // nq):
                        ot = op.tile([128, NQ], f16, tag="ot")
                        nc.vector.tensor_copy(
                            ot[:, :nq], ps[:, q * nq : (q + 1) * nq]
                        )
                        # alternate output rails (sync HWDGE ran ~105GB/s,
                        # scalar ~80GB/s; either alone barely keeps up)
                        eng = nc.scalar if (t * nhalves + h + q) % 2 else nc.sync
                        eng.dma_start(
                            out[
                                t * 128 : (t + 1) * 128,
                                h * NH + q * nq : h * NH + (q + 1) * nq,
                            ],
                            ot[:, :nq],
                        )
    nc.compile()
    return nc


# test.py reads these after a call for timing/trace introspection
last_results = None


def kernel(x, expert_indices, weights):
    x = np.asarray(x)
    ei = np.asarray(expert_indices)
    w = np.asarray(weights)
    M, K = x.shape
    E, K2, N = w.shape
    assert K == K2 and E == _NCORES

    counts = np.bincount(ei, minlength=E)
    T = max(1, -(-int(counts.max()) // 128))
    Mpad = T * 128
    order = np.argsort(ei, kind="stable")
    x_sorted = x[order]
    offs = np.zeros(E + 1, dtype=np.int64)
    np.cumsum(counts, out=offs[1:])

    in_maps = []
    for e in range(E):
        blk = x_sorted[offs[e] : offs[e + 1]]
        xeT = np.zeros((K, Mpad), dtype=np.float16)
        xeT[:, : blk.shape[0]] = blk.T
        in_maps.append({"xT": xeT, "w": np.ascontiguousarray(w[e])})

    nc = _build_program(T, K, N)

    from concourse.bass_utils import run_bass_kernel_spmd

    res = run_bass_kernel_spmd(nc, in_maps, list(range(E)))
    global last_results
    last_results = res

    out = np.empty((M, N), dtype=np.float16)
    for e in range(E):
        out[offs[e] : offs[e + 1]] = res.results[e]["out"][: counts[e]]
    return out


# revision 8
# speedup vs baseline: 1.2843x; 1.0073x over previous
"""MoE grouped-GEMM kernel for Trainium2 (8 NeuronCores, expert-parallel).

Problem: x [16384, 1024] fp16, expert_indices [16384] int32 (0..7),
weights [8, 1024, 4096] fp16. Output: fp16 [16384, 4096] in sorted-token
order (stable sort by expert), fp32 accumulation.

Sharding: the host performs the argsort/bincount dispatch (that IS the
sharding step) and gives core e the tokens routed to expert e as a
pre-transposed xT [K, Mpad] fp16 block plus that expert's weights
[K, N]. Every core runs the identical dense-GEMM program (token counts
padded to a common multiple of 128), so a single SPMD NEFF drives all 8
cores with no device-side collectives. The host concatenates the
per-expert output blocks, which is exactly sorted-token order.
"""

import numpy as np

_NCORES = 8


def _build_program(T, K, N):
    """Dense GEMM per core: out[Mpad, N] = xT.T @ w, fp32 PSUM accumulation.

    Layout per core:
      xT [K, Mpad] fp16  (x pre-transposed on host so K lands on partitions)
      w  [K, N]   fp16
      out [Mpad, N] fp16, Mpad = T*128

    PE mapping: stationary lhsT = xT k-tile [128, 128], moving rhs = w
    [128, 512] slice, PSUM [128m, 512n] fp32 accumulated over K/128
    k-tiles. PSUM is split into two 4-bank halves (bufs=2) so the
    DVE fp32->fp16 eviction of one half overlaps matmuls in the other.
    """
    from concourse import bacc, bass, tile
    import concourse.mybir as mybir

    f16 = mybir.dt.float16
    f32 = mybir.dt.float32
    Mpad = T * 128
    KT = K // 128            # k-tiles (contraction)
    NB = 512                 # one PSUM bank of fp32
    NH = 2048                # psum half (4 banks)
    nhalves = N // NH

    nc = bacc.Bacc(
        "TRN2", target_bir_lowering=False, debug=False, num_devices=_NCORES
    )
    xT = nc.dram_tensor("xT", [K, Mpad], f16, kind="ExternalInput").ap()
    w = nc.dram_tensor("w", [K, N], f16, kind="ExternalInput").ap()
    out = nc.dram_tensor("out", [Mpad, N], f16, kind="ExternalOutput").ap()

    with tile.TileContext(nc) as tc:
        with (
            tc.tile_pool(name="xw", bufs=1) as xw,
            tc.tile_pool(name="op", bufs=4) as op,
            tc.tile_pool(name="pp", bufs=2, space=bass.MemorySpace.PSUM) as pp,
        ):
            # Whole x and w stay SBUF-resident (~99KB/partition total).
            # Rail split: x strips ride gpsimd SWDGE, w strips ride the
            # sync HWDGE ring, outputs ride the scalar HWDGE ring, so
            # first-use loads are never FIFO'd behind later traffic.
            # h-outer loop order means phase h=0 only needs the first
            # 2048 w columns (4MB) to reach steady state; h=1 strips
            # stream in during the ~115us h=0 phase.
            # PE clock-gate warm-up: ~20 matmuls on memset tiles issued
            # during the initial DMA wait so the HAM un-throttles (1.2 ->
            # 2.4GHz takes ~3.4us of sustained PE activity) before the
            # first real matmul. The dummy psum slot is recycled by the
            # pool before any real accumulation starts.
            zs = xw.tile([128, 128], f16, tag="zstat")
            zm = xw.tile([128, NB], f16, tag="zmov")
            nc.vector.memset(zs[:], 0.0)
            nc.vector.memset(zm[:], 0.0)
            pwarm = pp.tile([128, NH], f32, tag="ps")
            for i in range(20):
                nc.tensor.matmul(
                    pwarm[:, 0:NB], zs[:], zm[:], start=(i == 0), stop=(i == 19)
                )

            xheads = []
            xrests = []
            ws = [[None] * nhalves for _ in range(KT)]
            # All input strips ride the sync HWDGE rail (it alone reaches
            # ~400GB/s; a second concurrent rail just splits HBM bandwidth
            # and delays first-use strips). Issue in exact first-use order:
            # tiny t=0 x heads + h0 w strips feed the first tile within
            # ~2us of the rail opening, x rests arrive before t=1, and the
            # h=1 strips aren't needed until ~115us in.
            for k in range(KT):
                xh = xw.tile([128, 128], f16, tag=f"xh{k}")
                nc.sync.dma_start(xh[:], xT[k * 128 : (k + 1) * 128, 0:128])
                xheads.append(xh)
                wt = xw.tile([128, NH], f16, tag=f"w{k}h0")
                nc.sync.dma_start(wt[:], w[k * 128 : (k + 1) * 128, 0:NH])
                ws[k][0] = wt
            for k in range(KT):
                xr = xw.tile([128, Mpad - 128], f16, tag=f"xr{k}")
                nc.sync.dma_start(xr[:], xT[k * 128 : (k + 1) * 128, 128:Mpad])
                xrests.append(xr)
            for h in range(1, nhalves):
                for k in range(KT):
                    wt = xw.tile([128, NH], f16, tag=f"w{k}h{h}")
                    nc.sync.dma_start(
                        wt[:], w[k * 128 : (k + 1) * 128, h * NH : (h + 1) * NH]
                    )
                    ws[k][h] = wt

            NQ = 1024  # output eviction chunk (cast + store pipelined)
            for h in range(nhalves):
                for t in range(T):
                    ps = pp.tile([128, NH], f32, tag="ps")
                    for k in range(KT):
                        if t == 0:
                            lhs = xheads[k][:]
                        else:
                            lhs = xrests[k][:, (t - 1) * 128 : t * 128]
                        for n in range(NH // NB):
                            nc.tensor.matmul(
                                ps[:, n * NB : (n + 1) * NB],
                                lhs,
                                ws[k][h][:, n * NB : (n + 1) * NB],
                                start=(k == 0),
                                stop=(k == KT - 1),
                            )
                    # finer chunks on the very last eviction shorten the tail
                    nq = 512 if (h == nhalves - 1 and t == T - 1) else NQ
                    for q in range(NH // nq):
                        ot = op.tile([128, NQ], f16, tag="ot")
                        nc.vector.tensor_copy(
                            ot[:, :nq], ps[:, q * nq : (q + 1) * nq]
                        )
                        # alternate output rails (sync HWDGE ran ~105GB/s,
                        # scalar ~80GB/s; either alone barely keeps up)
                        eng = nc.scalar if (t * nhalves + h + q) % 2 else nc.sync
                        eng.dma_start(
                            out[
                                t * 128 : (t + 1) * 128,
                                h * NH + q * nq : h * NH + (q + 1) * nq,
                            ],
                            ot[:, :nq],
                        )
    nc.compile()
    return nc


# test.py reads these after a call for timing/trace introspection
last_results = None


def kernel(x, expert_indices, weights):
    x = np.asarray(x)
    ei = np.asarray(expert_indices)
    w = np.asarray(weights)
    M, K = x.shape
    E, K2, N = w.shape
    assert K == K2 and E == _NCORES

    counts = np.bincount(ei, minlength=E)
    T = max(1, -(-int(counts.max()) // 128))
    Mpad = T * 128
    order = np.argsort(ei, kind="stable")
    x_sorted = x[order]
    offs = np.zeros(E + 1, dtype=np.int64)
    np.cumsum(counts, out=offs[1:])

    in_maps = []
    for e in range(E):
        blk = x_sorted[offs[e] : offs[e + 1]]
        xeT = np.zeros((K, Mpad), dtype=np.float16)
        xeT[:, : blk.shape[0]] = blk.T
        in_maps.append({"xT": xeT, "w": np.ascontiguousarray(w[e])})

    nc = _build_program(T, K, N)

    from concourse.bass_utils import run_bass_kernel_spmd

    res = run_bass_kernel_spmd(nc, in_maps, list(range(E)))
    global last_results
    last_results = res

    out = np.empty((M, N), dtype=np.float16)
    for e in range(E):
        out[offs[e] : offs[e + 1]] = res.results[e]["out"][: counts[e]]
    return out


# revision 9
# speedup vs baseline: 1.2972x; 1.0101x over previous
"""MoE grouped-GEMM kernel for Trainium2 (8 NeuronCores, expert-parallel).

Problem: x [16384, 1024] fp16, expert_indices [16384] int32 (0..7),
weights [8, 1024, 4096] fp16. Output: fp16 [16384, 4096] in sorted-token
order (stable sort by expert), fp32 accumulation.

Sharding: the host performs the argsort/bincount dispatch (that IS the
sharding step) and gives core e the tokens routed to expert e as a
pre-transposed xT [K, Mpad] fp16 block plus that expert's weights
[K, N]. Every core runs the identical dense-GEMM program (token counts
padded to a common multiple of 128), so a single SPMD NEFF drives all 8
cores with no device-side collectives. The host concatenates the
per-expert output blocks, which is exactly sorted-token order.
"""

import numpy as np

_NCORES = 8


def _build_program(T, K, N):
    """Dense GEMM per core: out[Mpad, N] = xT.T @ w, fp32 PSUM accumulation.

    Layout per core:
      xT [K, Mpad] fp16  (x pre-transposed on host so K lands on partitions)
      w  [K, N]   fp16
      out [Mpad, N] fp16, Mpad = T*128

    PE mapping: stationary lhsT = xT k-tile [128, 128], moving rhs = w
    [128, 512] slice, PSUM [128m, 512n] fp32 accumulated over K/128
    k-tiles. PSUM is split into two 4-bank halves (bufs=2) so the
    DVE fp32->fp16 eviction of one half overlaps matmuls in the other.
    """
    from concourse import bacc, bass, tile
    import concourse.mybir as mybir

    f16 = mybir.dt.float16
    f32 = mybir.dt.float32
    Mpad = T * 128
    KT = K // 128            # k-tiles (contraction)
    NB = 512                 # one PSUM bank of fp32
    NH = 2048                # psum half (4 banks)
    nhalves = N // NH

    nc = bacc.Bacc(
        "TRN2", target_bir_lowering=False, debug=False, num_devices=_NCORES
    )
    xT = nc.dram_tensor("xT", [K, Mpad], f16, kind="ExternalInput").ap()
    w = nc.dram_tensor("w", [K, N], f16, kind="ExternalInput").ap()
    out = nc.dram_tensor("out", [Mpad, N], f16, kind="ExternalOutput").ap()

    with tile.TileContext(nc) as tc:
        with (
            tc.tile_pool(name="xw", bufs=1) as xw,
            tc.tile_pool(name="op", bufs=4) as op,
            tc.tile_pool(name="pp", bufs=2, space=bass.MemorySpace.PSUM) as pp,
        ):
            # Whole x and w stay SBUF-resident (~99KB/partition total).
            # Rail split: x strips ride gpsimd SWDGE, w strips ride the
            # sync HWDGE ring, outputs ride the scalar HWDGE ring, so
            # first-use loads are never FIFO'd behind later traffic.
            # h-outer loop order means phase h=0 only needs the first
            # 2048 w columns (4MB) to reach steady state; h=1 strips
            # stream in during the ~115us h=0 phase.
            # PE clock-gate warm-up: ~20 matmuls on memset tiles issued
            # during the initial DMA wait so the HAM un-throttles (1.2 ->
            # 2.4GHz takes ~3.4us of sustained PE activity) before the
            # first real matmul. The dummy psum slot is recycled by the
            # pool before any real accumulation starts.
            zs = xw.tile([128, 128], f16, tag="zstat")
            zm = xw.tile([128, NB], f16, tag="zmov")
            nc.vector.memset(zs[:], 0.0)
            nc.vector.memset(zm[:], 0.0)
            pwarm = pp.tile([128, NH], f32, tag="ps")
            for i in range(10):
                nc.tensor.matmul(
                    pwarm[:, 0:NB], zs[:], zm[:], start=(i == 0), stop=(i == 9)
                )

            xheads = []
            xrests = []
            ws = [[None] * nhalves for _ in range(KT)]
            # All input strips ride the sync HWDGE rail (it alone reaches
            # ~400GB/s; a second concurrent rail just splits HBM bandwidth
            # and delays first-use strips). Issue in exact first-use order:
            # tiny t=0 x heads + h0 w strips feed the first tile within
            # ~2us of the rail opening, x rests arrive before t=1, and the
            # h=1 strips aren't needed until ~115us in.
            for k in range(KT):
                xh = xw.tile([128, 128], f16, tag=f"xh{k}")
                nc.sync.dma_start(xh[:], xT[k * 128 : (k + 1) * 128, 0:128])
                xheads.append(xh)
                wt = xw.tile([128, NH], f16, tag=f"w{k}h0")
                nc.sync.dma_start(wt[:], w[k * 128 : (k + 1) * 128, 0:NH])
                ws[k][0] = wt
            for k in range(KT):
                xr = xw.tile([128, Mpad - 128], f16, tag=f"xr{k}")
                nc.sync.dma_start(xr[:], xT[k * 128 : (k + 1) * 128, 128:Mpad])
                xrests.append(xr)
            for h in range(1, nhalves):
                for k in range(KT):
                    wt = xw.tile([128, NH], f16, tag=f"w{k}h{h}")
                    nc.sync.dma_start(
                        wt[:], w[k * 128 : (k + 1) * 128, h * NH : (h + 1) * NH]
                    )
                    ws[k][h] = wt

            NQ = 1024  # output eviction chunk (cast + store pipelined)
            for h in range(nhalves):
                for t in range(T):
                    ps = pp.tile([128, NH], f32, tag="ps")
                    for k in range(KT):
                        if t == 0:
                            lhs = xheads[k][:]
                        else:
                            lhs = xrests[k][:, (t - 1) * 128 : t * 128]
                        for n in range(NH // NB):
                            nc.tensor.matmul(
                                ps[:, n * NB : (n + 1) * NB],
                                lhs,
                                ws[k][h][:, n * NB : (n + 1) * NB],
                                start=(k == 0),
                                stop=(k == KT - 1),
                            )
                    # finer chunks on the very last eviction shorten the tail
                    nq = 512 if (h == nhalves - 1 and t == T - 1) else NQ
                    for q in range(NH // nq):
                        ot = op.tile([128, NQ], f16, tag="ot")
                        nc.vector.tensor_copy(
                            ot[:, :nq], ps[:, q * nq : (q + 1) * nq]
                        )
                        # alternate output rails (sync HWDGE ran ~105GB/s,
                        # scalar ~80GB/s; either alone barely keeps up)
                        eng = nc.scalar if (t * nhalves + h + q) % 2 else nc.sync
                        eng.dma_start(
                            out[
                                t * 128 : (t + 1) * 128,
                                h * NH + q * nq : h * NH + (q + 1) * nq,
                            ],
                            ot[:, :nq],
                        )
    nc.compile()
    return nc


# test.py reads these after a call for timing/trace introspection
last_results = None


def kernel(x, expert_indices, weights):
    x = np.asarray(x)
    ei = np.asarray(expert_indices)
    w = np.asarray(weights)
    M, K = x.shape
    E, K2, N = w.shape
    assert K == K2 and E == _NCORES

    counts = np.bincount(ei, minlength=E)
    T = max(1, -(-int(counts.max()) // 128))
    Mpad = T * 128
    order = np.argsort(ei, kind="stable")
    x_sorted = x[order]
    offs = np.zeros(E + 1, dtype=np.int64)
    np.cumsum(counts, out=offs[1:])

    in_maps = []
    for e in range(E):
        blk = x_sorted[offs[e] : offs[e + 1]]
        xeT = np.zeros((K, Mpad), dtype=np.float16)
        xeT[:, : blk.shape[0]] = blk.T
        in_maps.append({"xT": xeT, "w": np.ascontiguousarray(w[e])})

    nc = _build_program(T, K, N)

    from concourse.bass_utils import run_bass_kernel_spmd

    res = run_bass_kernel_spmd(nc, in_maps, list(range(E)))
    global last_results
    last_results = res

    out = np.empty((M, N), dtype=np.float16)
    for e in range(E):
        out[offs[e] : offs[e + 1]] = res.results[e]["out"][: counts[e]]
    return out
